# revision 9
# baseline (speedup 1.0000x reference)
"""Trainium2 Bass kernel for nn_Attention_64235530879146 (v3).

Per core (B=1, C=512, T=1024, 8 heads of ch=64, 32 groups):
    xn = GroupNorm(x) * gn_weight + gn_bias          # [C, T]
    qkv = W1 @ xn + b1                               # [3C, T]
    per head: St[s,t] = (k*sc)^T (q*sc),  sc = ch**-0.25
              Wt = exp(St);  a = (V Wt) / r,  r[t] = sum_s Wt[s,t]
    out = a + x

Sharding: pure data-parallel over batch (8 elements on 8 cores).

Engine plan: ACT (exp) and DVE (psum evacuation + softmax normalize)
are the co-bottlenecks (~60-70us each); PE (~65us bf16) hides under
them once warm; GPSIMD carries broadcasts/residual-adds/DMA issue.

  - PE warmup burst at t=0 (HAM un-throttle) so all real matmuls run
    at 2.4 GHz.
  - x loads split across 4 DMA queues; GroupNorm stats per-tile as
    they land; rstd via Newton rsqrt on DVE (only the exp ACT table is
    ever loaded, preloaded during the DMA phase by a dummy).
  - Head-pair pipeline: pair j's scores/exp/AV stream (ACT-paced)
    while q/k for pair j+1 and (pair 0) the V chunks trickle through a
    1-bank staging slot.
  - PSUM: scores 2x[128,1024] (4 banks), av 3x[65,512], staging 1.
  - AV keeps the ones-column (M=65) so r rides in psum row 64:
    1/r = reciprocal_approx_fast straight off psum, broadcast to 64
    partitions by gpsimd.partition_broadcast (no DMA round trips).
  - v-bias folds into the epilogue via sum_s b1v*Wt = b1v*r:
    out = (av/r + b1v) + x (scalar_tensor_tensor, split DVE/gpsimd).
  - Last pair runs all 4 AV accumulators inline (4th av bank borrowed
    from the idle staging slot) so there is no deferred tail.
  - DVE_EXP_UNITS: those (st, n) score units skip ACT; DVE computes
    exp via the Schraudolph int16-exponent trick (tensor_scalar into a
    bf16-bitcast tile). Softmax renormalization cancels the trick's
    constant bias; tolerance 2e-2 absorbs the +-3% mantissa ripple.
"""
import numpy as np

GROUPS = 32
HEADS = 8
EPS = 1e-5
C = 512
T = 1024
CH = C // HEADS            # 64
SCALE = float(CH) ** -0.25
N_CORES = 8

DVE_EXP_UNITS = set()   # (st, n) units whose exp runs on DVE
N_WARM = 32                # PE warmup matmuls (HAM un-throttle)

LOG2E_128 = 184.6650558    # log2(e) * 128  (bf16 exponent scale)
SCHRAUD_B = 16250.3        # 127*128 - 5.7 (centres the 2^frac error band)


def _build_nc():
    import concourse.bass as bass
    import concourse.mybir as mybir
    import concourse.tile as tile
    from concourse import bacc

    f32 = mybir.dt.float32
    bf16 = mybir.dt.bfloat16
    i16 = mybir.dt.int16
    i32 = mybir.dt.int32
    Alu = mybir.AluOpType
    Act = mybir.ActivationFunctionType

    nc = bacc.Bacc("TRN2", target_bir_lowering=False, debug=False)

    x_d = nc.declare_dram_parameter("x", [C, T], f32, isOutput=False)
    w1t_d = nc.declare_dram_parameter("w1t", [C, 3 * C], bf16, isOutput=False)
    b1r_d = nc.declare_dram_parameter("b1r", [128, 8], f32, isOutput=False)
    b1vh_d = nc.declare_dram_parameter("b1vh", [64, 8], f32, isOutput=False)
    gnw_d = nc.declare_dram_parameter("gnw", [128, 4], f32, isOutput=False)
    gnb_d = nc.declare_dram_parameter("gnb", [128, 4], f32, isOutput=False)
    ind16_d = nc.declare_dram_parameter("ind16", [128, 8], f32, isOutput=False)
    indT_d = nc.declare_dram_parameter("indT", [8, 128], f32, isOutput=False)
    out_d = nc.declare_dram_parameter("out", [C, T], f32, isOutput=True)

    with tile.TileContext(nc) as tc:
        with (
            tc.tile_pool(name="cst", bufs=1) as cst,
            tc.tile_pool(name="work", bufs=2) as work,
            tc.tile_pool(name="wtp", bufs=4) as wtp,
            tc.tile_pool(name="outp", bufs=4) as outp,
            tc.tile_pool(name="ps", bufs=1, space="PSUM") as ps,
        ):
            # ---------------- PE warmup (no data deps) ----------------
            wuA = cst.tile([128, 128], bf16)
            nc.vector.memset(wuA, 0.0)
            wuB = cst.tile([128, 512], bf16)
            nc.vector.memset(wuB, 0.0)
            wu_ps = ps.tile([128, 512], f32, tag="sc", bufs=2, name="warm")
            for i in range(N_WARM):
                nc.tensor.matmul(
                    out=wu_ps, lhsT=wuA, rhs=wuB, start=True, stop=True
                )

            # ---------------- loads ----------------
            # tiny consts first on the gpsimd queue
            b1r_sb = cst.tile([128, 8], f32)
            nc.gpsimd.dma_start(out=b1r_sb, in_=b1r_d[:, :])
            b1vh_sb = cst.tile([64, 8], f32)
            nc.gpsimd.dma_start(out=b1vh_sb, in_=b1vh_d[:, :])
            gnw_sb = cst.tile([128, 4], f32)
            nc.gpsimd.dma_start(out=gnw_sb, in_=gnw_d[:, :])
            gnb_sb = cst.tile([128, 4], f32)
            nc.gpsimd.dma_start(out=gnb_sb, in_=gnb_d[:, :])
            ind16 = cst.tile([128, 8], f32)
            nc.gpsimd.dma_start(out=ind16, in_=ind16_d[:, :])
            indT = cst.tile([8, 128], f32)
            nc.gpsimd.dma_start(out=indT, in_=indT_d[:, :])

            # x: one c-tile per queue so GroupNorm can start ASAP
            xv = x_d.ap().rearrange("(i p) t -> i p t", p=128)
            x_sb = cst.tile([128, 4, T], f32)
            nc.sync.dma_start(out=x_sb[:, 0, :], in_=xv[0])
            nc.scalar.dma_start(out=x_sb[:, 1, :], in_=xv[1])
            nc.sync.dma_start(out=x_sb[:, 2, :], in_=xv[2])
            nc.scalar.dma_start(out=x_sb[:, 3, :], in_=xv[3])

            w1t_sb = cst.tile([128, 4, 3 * C], bf16)
            w1tv = w1t_d.ap().rearrange("(i p) o -> p i o", p=128)
            nc.scalar.dma_start(out=w1t_sb[:, :, 0:C], in_=w1tv[:, :, 0:C])
            nc.scalar.dma_start(out=w1t_sb[:, :, C : 2 * C], in_=w1tv[:, :, C : 2 * C])
            nc.sync.dma_start(out=w1t_sb[:, :, 2 * C :], in_=w1tv[:, :, 2 * C :])

            # head-aligned residual copy of x (needed ~20us in)
            x_hd = cst.tile([64, 8, T], f32)
            nc.gpsimd.dma_start(out=x_hd, in_=x_d.ap().rearrange("(h p) t -> p h t", p=64))

            # preload the exp table while DMAs stream
            dumm = cst.tile([8, 2], f32)
            nc.vector.memset(dumm, 0.0)
            nc.scalar.activation(
                out=dumm[:, 1:2], in_=dumm[:, 0:1], func=Act.Exp, bias=0.0, scale=1.0
            )

            # ---------------- GroupNorm ----------------
            rhs3 = cst.tile([128, 4, 3], f32)
            for i in range(4):
                st6 = work.tile([128, 2, 6], f32, tag="st6")
                nc.vector.bn_stats(out=st6[:, 0, :], in_=x_sb[:, i, 0:512])
                nc.vector.bn_stats(out=st6[:, 1, :], in_=x_sb[:, i, 512:1024])
                mv = work.tile([128, 2], f32, tag="mv")
                nc.vector.bn_aggr(out=mv, in_=st6)
                nc.vector.tensor_copy(out=rhs3[:, i, 0:2], in_=mv)
                nc.vector.tensor_mul(rhs3[:, i, 2:3], mv[:, 0:1], mv[:, 0:1])

            stats_ps = ps.tile([8, 12], f32, tag="qkv", name="stats_ps")
            for i in range(4):
                nc.tensor.matmul(
                    out=stats_ps[:, 3 * i : 3 * i + 3],
                    lhsT=ind16,
                    rhs=rhs3[:, i, :],
                    start=True,
                    stop=True,
                )
            sg = cst.tile([8, 12], f32)
            nc.vector.tensor_copy(out=sg, in_=stats_ps)
            musig = cst.tile([8, 2, 4], f32)
            mu_v = sg.rearrange("p (i three) -> p i three", three=3)
            nc.vector.tensor_copy(out=musig[:, 0, :], in_=mu_v[:, :, 0])
            var_g = cst.tile([8, 4], f32)
            nc.vector.tensor_add(var_g, mu_v[:, :, 1], mu_v[:, :, 2])
            mu2 = cst.tile([8, 4], f32)
            nc.vector.tensor_mul(mu2, mu_v[:, :, 0], mu_v[:, :, 0])
            nc.vector.tensor_sub(var_g, var_g, mu2)
            # rstd = 1/sqrt(var+eps): bit-trick seed + 2 Newton steps (DVE)
            ve = cst.tile([8, 4], f32)
            nc.vector.tensor_scalar(
                out=ve, in0=var_g, scalar1=EPS, scalar2=None, op0=Alu.add
            )
            t_i = cst.tile([8, 4], i32)
            nc.vector.tensor_scalar(
                out=t_i, in0=ve.bitcast(i32), scalar1=1, scalar2=None,
                op0=Alu.logical_shift_right,
            )
            t_x = cst.tile([8, 4], i32)
            nc.vector.tensor_scalar(
                out=t_x, in0=t_i, scalar1=-1, scalar2=None, op0=Alu.bitwise_xor
            )
            y_i = cst.tile([8, 4], i32)
            nc.vector.tensor_scalar(
                out=y_i, in0=t_x, scalar1=0x5F3759E0, scalar2=None, op0=Alu.add
            )
            y0 = y_i.bitcast(f32)
            t2 = cst.tile([8, 4], f32)
            nc.vector.tensor_mul(t2, y0, y0)
            nc.vector.tensor_mul(t2, t2, ve)
            nc.vector.tensor_scalar(
                out=t2, in0=t2, scalar1=-0.5, scalar2=1.5, op0=Alu.mult, op1=Alu.add
            )
            y1 = cst.tile([8, 4], f32)
            nc.vector.tensor_mul(y1, y0, t2)
            t3 = cst.tile([8, 4], f32)
            nc.vector.tensor_mul(t3, y1, y1)
            nc.vector.tensor_mul(t3, t3, ve)
            nc.vector.tensor_scalar(
                out=t3, in0=t3, scalar1=-0.5, scalar2=1.5, op0=Alu.mult, op1=Alu.add
            )
            nc.vector.tensor_mul(musig[:, 1, :], y1, t3)

            # broadcast (mu, rstd) to channels; fold gn affine; xn bf16
            xn_sb = cst.tile([128, 4, T], bf16)
            af = cst.tile([128, 4, 2], f32)
            for i in range(4):
                musig_ps = ps.tile([128, 2], f32, tag="qkv", name=f"musig_ps{i}")
                nc.tensor.matmul(
                    out=musig_ps, lhsT=indT, rhs=musig[:, :, i], start=True, stop=True
                )
                nc.vector.tensor_mul(af[:, i, 0:1], gnw_sb[:, i : i + 1], musig_ps[:, 1:2])
                tmp = work.tile([128, 1], f32, tag="tmp1")
                nc.vector.tensor_mul(tmp, musig_ps[:, 0:1], af[:, i, 0:1])
                nc.vector.tensor_sub(af[:, i, 1:2], gnb_sb[:, i : i + 1], tmp)
                nc.vector.tensor_scalar(
                    out=xn_sb[:, i, :],
                    in0=x_sb[:, i, :],
                    scalar1=af[:, i, 0:1],
                    scalar2=af[:, i, 1:2],
                    op0=Alu.mult,
                    op1=Alu.add,
                )

            # ---------------- QKV machinery ----------------
            q_sb = cst.tile([128, 4, T], bf16)
            k_sb = cst.tile([128, 4, T], bf16)
            vt_sb = cst.tile([128, 8, 8, 65], bf16)
            nc.vector.memset(vt_sb[:, :, :, 64:65], 1.0)

            def emit_qk_boot(j):
                # prologue q_j/k_j through the sc slots, full width
                for oc, dst in ((j, q_sb), (4 + j, k_sb)):
                    qp = ps.tile([128, T], f32, tag="sc", bufs=2, name=f"boot_{oc}")
                    for n in range(2):
                        for i in range(4):
                            nc.tensor.matmul(
                                out=qp[:, 512 * n : 512 * n + 512],
                                lhsT=w1t_sb[:, i, 128 * oc : 128 * oc + 128],
                                rhs=xn_sb[:, i, 512 * n : 512 * n + 512],
                                start=(i == 0),
                                stop=(i == 3),
                            )
                    nc.vector.tensor_scalar(
                        out=dst[:, j, :],
                        in0=qp,
                        scalar1=b1r_sb[:, oc : oc + 1],
                        scalar2=SCALE,
                        op0=Alu.add,
                        op1=Alu.mult,
                    )

            def emit_qk(j):
                # steady-state q_j/k_j in [128,512] halves through the staging slot
                for oc, dst in ((j, q_sb), (4 + j, k_sb)):
                    for n in range(2):
                        qp = ps.tile([128, 512], f32, tag="qkv", name=f"qk_{oc}_{n}")
                        for i in range(4):
                            nc.tensor.matmul(
                                out=qp,
                                lhsT=w1t_sb[:, i, 128 * oc : 128 * oc + 128],
                                rhs=xn_sb[:, i, 512 * n : 512 * n + 512],
                                start=(i == 0),
                                stop=(i == 3),
                            )
                        nc.vector.tensor_scalar(
                            out=dst[:, j, 512 * n : 512 * n + 512],
                            in0=qp,
                            scalar1=b1r_sb[:, oc : oc + 1],
                            scalar2=SCALE,
                            op0=Alu.add,
                            op1=Alu.mult,
                        )

            def emit_v(st):
                vp = ps.tile([128, 512], f32, tag="qkv", name=f"v_{st}")
                for i in range(4):
                    nc.tensor.matmul(
                        out=vp,
                        lhsT=xn_sb[:, i, 128 * st : 128 * st + 128],
                        rhs=w1t_sb[:, i, 2 * C : 3 * C],
                        start=(i == 0),
                        stop=(i == 3),
                    )
                nc.vector.tensor_copy(
                    out=vt_sb[:, st, :, 0:64],
                    in_=vp.rearrange("p (h c) -> p h c", c=64),
                )

            def epilogue(j, h, n, avt):
                # 1/r straight off psum row 64; gpsimd broadcasts it
                rrowi = wtp.tile([1, 512], f32, tag="rrowi", bufs=4, name=f"ri_{h}_{n}")
                nc.vector.reciprocal_approx_fast(out=rrowi, in_=avt[64:65, :])
                rbc = wtp.tile([64, 512], f32, tag="rbc", bufs=4, name=f"rbc_{h}_{n}")
                nc.gpsimd.partition_broadcast(rbc, rrowi)
                o_bf = outp.tile([64, 512], bf16, tag="obf", name=f"ob_{h}_{n}")
                nc.vector.tensor_mul(o_bf, avt[0:64, :], rbc)
                out_f = outp.tile([64, 512], f32, tag="of", name=f"of_{h}_{n}")
                if n == 0:
                    ob2 = outp.tile([64, 512], bf16, tag="ob2", name=f"o2_{h}_{n}")
                    nc.gpsimd.tensor_scalar(
                        out=ob2, in0=o_bf, scalar1=b1vh_sb[:, h : h + 1],
                        scalar2=None, op0=Alu.add,
                    )
                    nc.gpsimd.tensor_add(
                        out_f, ob2, x_hd[:, h, 512 * n : 512 * n + 512]
                    )
                else:
                    nc.vector.scalar_tensor_tensor(
                        out=out_f,
                        in0=o_bf,
                        scalar=b1vh_sb[:, h : h + 1],
                        in1=x_hd[:, h, 512 * n : 512 * n + 512],
                        op0=Alu.add,
                        op1=Alu.add,
                    )
                nc.gpsimd.dma_start(
                    out=out_d[64 * h : 64 * h + 64, 512 * n : 512 * n + 512], in_=out_f
                )

            # ---------------- pipeline ----------------
            emit_qk_boot(0)

            for j in range(HEADS // 2):
                hA, hB = 2 * j, 2 * j + 1
                last = j == HEADS // 2 - 1
                av = {
                    (hA, 0): ps.tile([65, 512], f32, tag="av", bufs=3, name=f"av_{hA}_0"),
                    (hB, 0): ps.tile([65, 512], f32, tag="av", bufs=3, name=f"av_{hB}_0"),
                }
                if last:
                    # borrow the idle staging slot for a 4th inline accumulator
                    av[(hA, 1)] = ps.tile([128, 512], f32, tag="qkv", name=f"av_{hA}_1")
                    av[(hB, 1)] = ps.tile([65, 512], f32, tag="av", bufs=3, name=f"av_{hB}_1")
                wts = []
                for st in range(8):
                    if j == 0:
                        emit_v(st)
                    wt_pair = []
                    for n in range(2):
                        scn = ps.tile([128, T], f32, tag="sc", bufs=2, name=f"sc_{j}_{st}_{n}")
                        for hi, h in enumerate((hA, hB)):
                            hp = 64 * hi
                            nc.tensor.matmul(
                                out=scn[:, 512 * hi : 512 * hi + 512],
                                lhsT=k_sb[hp : hp + 64, j, 128 * st : 128 * st + 128],
                                rhs=q_sb[hp : hp + 64, j, 512 * n : 512 * n + 512],
                                start=True,
                                stop=True,
                                tile_position=(hp, 0),
                            )
                        wtn = wtp.tile(
                            [128, T], bf16, tag="wt", bufs=20, name=f"wt_{j}_{st}_{n}"
                        )
                        if (st, n) in DVE_EXP_UNITS:
                            nc.vector.tensor_scalar(
                                out=wtn.bitcast(i16),
                                in0=scn,
                                scalar1=LOG2E_128,
                                scalar2=SCHRAUD_B,
                                op0=Alu.mult,
                                op1=Alu.add,
                            )
                        else:
                            nc.scalar.activation(
                                out=wtn, in_=scn, func=Act.Exp, bias=0.0, scale=1.0
                            )
                        wt_pair.append(wtn)
                    wts.append(wt_pair)

                    # AV (n=0 inline; n=1 too on the last pair)
                    n_range = (0, 1) if last else (0,)
                    for n in n_range:
                        for hi, h in enumerate((hA, hB)):
                            nc.tensor.matmul(
                                out=av[(h, n)][0:65, :],
                                lhsT=vt_sb[:, st, h, 0:65],
                                rhs=wt_pair[n][:, 512 * hi : 512 * hi + 512],
                                start=(st == 0),
                                stop=(st == 7),
                            )

                if not last:
                    emit_qk(j + 1)
                    epilogue(j, hA, 0, av[(hA, 0)])
                    av[(hA, 1)] = ps.tile([65, 512], f32, tag="av", bufs=3, name=f"av_{hA}_1")
                    epilogue(j, hB, 0, av[(hB, 0)])
                    av[(hB, 1)] = ps.tile([65, 512], f32, tag="av", bufs=3, name=f"av_{hB}_1")
                    for st in range(8):
                        for hi, h in enumerate((hA, hB)):
                            nc.tensor.matmul(
                                out=av[(h, 1)][0:65, :],
                                lhsT=vt_sb[:, st, h, 0:65],
                                rhs=wts[st][1][:, 512 * hi : 512 * hi + 512],
                                start=(st == 0),
                                stop=(st == 7),
                            )
                    epilogue(j, hA, 1, av[(hA, 1)])
                    epilogue(j, hB, 1, av[(hB, 1)])
                else:
                    for h in (hA, hB):
                        for n in (0, 1):
                            epilogue(j, h, n, av[(h, n)])

    nc.finalize()
    return nc


def _make_in_maps(inputs):
    x = np.ascontiguousarray(np.asarray(inputs["x"], dtype=np.float32))
    gnw = np.asarray(inputs["gn_weight"], dtype=np.float32)
    gnb = np.asarray(inputs["gn_bias"], dtype=np.float32)
    w1 = np.asarray(inputs["w1"], dtype=np.float32)
    b1 = np.asarray(inputs["b1"], dtype=np.float32)

    import ml_dtypes

    B = x.shape[0]
    w1t = np.ascontiguousarray(w1[:, :, 0].T).astype(ml_dtypes.bfloat16)  # [C, 3C]
    b1r = np.ascontiguousarray(b1[: 2 * C].reshape(8, 128).T)       # [128, 8]
    b1vh = np.ascontiguousarray(b1[2 * C :].reshape(8, 64).T)       # [64, 8]
    gnw_r = np.ascontiguousarray(gnw.reshape(4, 128).T)             # [128, 4]
    gnb_r = np.ascontiguousarray(gnb.reshape(4, 128).T)             # [128, 4]

    ind16 = np.zeros((128, 8), np.float32)
    indT = np.zeros((8, 128), np.float32)
    for g in range(8):
        ind16[16 * g : 16 * g + 16, g] = 1.0 / 16.0
        indT[g, 16 * g : 16 * g + 16] = 1.0

    in_maps = []
    for b in range(B):
        in_maps.append(
            {
                "x": np.ascontiguousarray(x[b].reshape(C, T)),
                "w1t": w1t,
                "b1r": b1r,
                "b1vh": b1vh,
                "gnw": gnw_r,
                "gnb": gnb_r,
                "ind16": ind16,
                "indT": indT,
            }
        )
    return in_maps


def _gather(results, x_shape):
    B, Cc, H, W = x_shape
    out = np.empty((B, Cc, H, W), dtype=np.float32)
    for b in range(B):
        out[b] = results[b]["out"].reshape(Cc, H, W)
    return out


def kernel(**inputs):
    from concourse.bass_utils import run_bass_kernel_spmd

    nc = _build_nc()
    in_maps = _make_in_maps(inputs)
    res = run_bass_kernel_spmd(nc, in_maps, core_ids=list(range(N_CORES)))
    return _gather(res.results, np.asarray(inputs["x"]).shape)


# revision 11
# speedup vs baseline: 1.3440x; 1.3440x over previous
"""Trainium2 Bass kernel for nn_Attention_64235530879146 (v3).

Per core (B=1, C=512, T=1024, 8 heads of ch=64, 32 groups):
    xn = GroupNorm(x) * gn_weight + gn_bias          # [C, T]
    qkv = W1 @ xn + b1                               # [3C, T]
    per head: St[s,t] = (k*sc)^T (q*sc),  sc = ch**-0.25
              Wt = exp(St);  a = (V Wt) / r,  r[t] = sum_s Wt[s,t]
    out = a + x

Sharding: pure data-parallel over batch (8 elements on 8 cores).

Engine plan: ACT (exp) and DVE (psum evacuation + softmax normalize)
are the co-bottlenecks (~60-70us each); PE (~65us bf16) hides under
them once warm; GPSIMD carries broadcasts/residual-adds/DMA issue.

  - PE warmup burst at t=0 (HAM un-throttle) so all real matmuls run
    at 2.4 GHz.
  - x loads split across 4 DMA queues; GroupNorm stats per-tile as
    they land; rstd via Newton rsqrt on DVE (only the exp ACT table is
    ever loaded, preloaded during the DMA phase by a dummy).
  - Head-pair pipeline: pair j's scores/exp/AV stream (ACT-paced)
    while q/k for pair j+1 and (pair 0) the V chunks trickle through a
    1-bank staging slot.
  - PSUM: scores 2x[128,1024] (4 banks), av 3x[65,512], staging 1.
  - AV keeps the ones-column (M=65) so r rides in psum row 64:
    1/r = reciprocal_approx_fast straight off psum, broadcast to 64
    partitions by gpsimd.partition_broadcast (no DMA round trips).
  - v-bias folds into the epilogue via sum_s b1v*Wt = b1v*r:
    out = (av/r + b1v) + x (scalar_tensor_tensor, split DVE/gpsimd).
  - Last pair runs all 4 AV accumulators inline (4th av bank borrowed
    from the idle staging slot) so there is no deferred tail.
  - DVE_EXP_UNITS: those (st, n) score units skip ACT; DVE computes
    exp via the Schraudolph int16-exponent trick (tensor_scalar into a
    bf16-bitcast tile). Softmax renormalization cancels the trick's
    constant bias; tolerance 2e-2 absorbs the +-3% mantissa ripple.
"""
import numpy as np

GROUPS = 32
HEADS = 8
EPS = 1e-5
C = 512
T = 1024
CH = C // HEADS            # 64
SCALE = float(CH) ** -0.25
N_CORES = 8

DVE_EXP_UNITS = set()   # (st, n) units whose exp runs on DVE
EPI_V2 = True              # bisect: v2-style epilogue (DMA r-chain)
LAST_INLINE = False        # bisect: v2-style deferred last pair
N_WARM = 32                # PE warmup matmuls (HAM un-throttle)

LOG2E_128 = 184.6650558    # log2(e) * 128  (bf16 exponent scale)
SCHRAUD_B = 16250.3        # 127*128 - 5.7 (centres the 2^frac error band)


def _build_nc():
    import concourse.bass as bass
    import concourse.mybir as mybir
    import concourse.tile as tile
    from concourse import bacc

    f32 = mybir.dt.float32
    bf16 = mybir.dt.bfloat16
    i16 = mybir.dt.int16
    i32 = mybir.dt.int32
    Alu = mybir.AluOpType
    Act = mybir.ActivationFunctionType

    nc = bacc.Bacc("TRN2", target_bir_lowering=False, debug=False)

    x_d = nc.declare_dram_parameter("x", [C, T], f32, isOutput=False)
    w1t_d = nc.declare_dram_parameter("w1t", [C, 3 * C], bf16, isOutput=False)
    b1r_d = nc.declare_dram_parameter("b1r", [128, 8], f32, isOutput=False)
    b1vh_d = nc.declare_dram_parameter("b1vh", [64, 8], f32, isOutput=False)
    gnw_d = nc.declare_dram_parameter("gnw", [128, 4], f32, isOutput=False)
    gnb_d = nc.declare_dram_parameter("gnb", [128, 4], f32, isOutput=False)
    ind16_d = nc.declare_dram_parameter("ind16", [128, 8], f32, isOutput=False)
    indT_d = nc.declare_dram_parameter("indT", [8, 128], f32, isOutput=False)
    out_d = nc.declare_dram_parameter("out", [C, T], f32, isOutput=True)

    with tile.TileContext(nc) as tc:
        with (
            tc.tile_pool(name="cst", bufs=1) as cst,
            tc.tile_pool(name="work", bufs=2) as work,
            tc.tile_pool(name="wtp", bufs=4) as wtp,
            tc.tile_pool(name="outp", bufs=4) as outp,
            tc.tile_pool(name="ps", bufs=1, space="PSUM") as ps,
        ):
            # ---------------- PE warmup (no data deps) ----------------
            wuA = cst.tile([128, 128], bf16)
            nc.vector.memset(wuA, 0.0)
            wuB = cst.tile([128, 512], bf16)
            nc.vector.memset(wuB, 0.0)
            wu_ps = ps.tile([128, 512], f32, tag="sc", bufs=2, name="warm")
            for i in range(N_WARM):
                nc.tensor.matmul(
                    out=wu_ps, lhsT=wuA, rhs=wuB, start=True, stop=True
                )

            # ---------------- loads ----------------
            # tiny consts first on the gpsimd queue
            b1r_sb = cst.tile([128, 8], f32)
            nc.gpsimd.dma_start(out=b1r_sb, in_=b1r_d[:, :])
            b1vh_sb = cst.tile([64, 8], f32)
            nc.gpsimd.dma_start(out=b1vh_sb, in_=b1vh_d[:, :])
            gnw_sb = cst.tile([128, 4], f32)
            nc.gpsimd.dma_start(out=gnw_sb, in_=gnw_d[:, :])
            gnb_sb = cst.tile([128, 4], f32)
            nc.gpsimd.dma_start(out=gnb_sb, in_=gnb_d[:, :])
            ind16 = cst.tile([128, 8], f32)
            nc.gpsimd.dma_start(out=ind16, in_=ind16_d[:, :])
            indT = cst.tile([8, 128], f32)
            nc.gpsimd.dma_start(out=indT, in_=indT_d[:, :])

            # x: one c-tile per queue so GroupNorm can start ASAP
            xv = x_d.ap().rearrange("(i p) t -> i p t", p=128)
            x_sb = cst.tile([128, 4, T], f32)
            nc.sync.dma_start(out=x_sb[:, 0, :], in_=xv[0])
            nc.scalar.dma_start(out=x_sb[:, 1, :], in_=xv[1])
            nc.sync.dma_start(out=x_sb[:, 2, :], in_=xv[2])
            nc.scalar.dma_start(out=x_sb[:, 3, :], in_=xv[3])

            w1t_sb = cst.tile([128, 4, 3 * C], bf16)
            w1tv = w1t_d.ap().rearrange("(i p) o -> p i o", p=128)
            nc.scalar.dma_start(out=w1t_sb[:, :, 0:C], in_=w1tv[:, :, 0:C])
            nc.scalar.dma_start(out=w1t_sb[:, :, C : 2 * C], in_=w1tv[:, :, C : 2 * C])
            nc.sync.dma_start(out=w1t_sb[:, :, 2 * C :], in_=w1tv[:, :, 2 * C :])

            # head-aligned residual copy of x (needed ~20us in)
            x_hd = cst.tile([64, 8, T], f32)
            nc.gpsimd.dma_start(out=x_hd, in_=x_d.ap().rearrange("(h p) t -> p h t", p=64))

            # preload the exp table while DMAs stream
            dumm = cst.tile([8, 2], f32)
            nc.vector.memset(dumm, 0.0)
            nc.scalar.activation(
                out=dumm[:, 1:2], in_=dumm[:, 0:1], func=Act.Exp, bias=0.0, scale=1.0
            )

            # ---------------- GroupNorm ----------------
            rhs3 = cst.tile([128, 4, 3], f32)
            for i in range(4):
                st6 = work.tile([128, 2, 6], f32, tag="st6")
                nc.vector.bn_stats(out=st6[:, 0, :], in_=x_sb[:, i, 0:512])
                nc.vector.bn_stats(out=st6[:, 1, :], in_=x_sb[:, i, 512:1024])
                mv = work.tile([128, 2], f32, tag="mv")
                nc.vector.bn_aggr(out=mv, in_=st6)
                nc.vector.tensor_copy(out=rhs3[:, i, 0:2], in_=mv)
                nc.vector.tensor_mul(rhs3[:, i, 2:3], mv[:, 0:1], mv[:, 0:1])

            stats_ps = ps.tile([8, 12], f32, tag="qkv", name="stats_ps")
            for i in range(4):
                nc.tensor.matmul(
                    out=stats_ps[:, 3 * i : 3 * i + 3],
                    lhsT=ind16,
                    rhs=rhs3[:, i, :],
                    start=True,
                    stop=True,
                )
            sg = cst.tile([8, 12], f32)
            nc.vector.tensor_copy(out=sg, in_=stats_ps)
            musig = cst.tile([8, 2, 4], f32)
            mu_v = sg.rearrange("p (i three) -> p i three", three=3)
            nc.vector.tensor_copy(out=musig[:, 0, :], in_=mu_v[:, :, 0])
            var_g = cst.tile([8, 4], f32)
            nc.vector.tensor_add(var_g, mu_v[:, :, 1], mu_v[:, :, 2])
            mu2 = cst.tile([8, 4], f32)
            nc.vector.tensor_mul(mu2, mu_v[:, :, 0], mu_v[:, :, 0])
            nc.vector.tensor_sub(var_g, var_g, mu2)
            # rstd = 1/sqrt(var+eps): bit-trick seed + 2 Newton steps (DVE)
            ve = cst.tile([8, 4], f32)
            nc.vector.tensor_scalar(
                out=ve, in0=var_g, scalar1=EPS, scalar2=None, op0=Alu.add
            )
            t_i = cst.tile([8, 4], i32)
            nc.vector.tensor_scalar(
                out=t_i, in0=ve.bitcast(i32), scalar1=1, scalar2=None,
                op0=Alu.logical_shift_right,
            )
            t_x = cst.tile([8, 4], i32)
            nc.vector.tensor_scalar(
                out=t_x, in0=t_i, scalar1=-1, scalar2=None, op0=Alu.bitwise_xor
            )
            y_i = cst.tile([8, 4], i32)
            nc.vector.tensor_scalar(
                out=y_i, in0=t_x, scalar1=0x5F3759E0, scalar2=None, op0=Alu.add
            )
            y0 = y_i.bitcast(f32)
            t2 = cst.tile([8, 4], f32)
            nc.vector.tensor_mul(t2, y0, y0)
            nc.vector.tensor_mul(t2, t2, ve)
            nc.vector.tensor_scalar(
                out=t2, in0=t2, scalar1=-0.5, scalar2=1.5, op0=Alu.mult, op1=Alu.add
            )
            y1 = cst.tile([8, 4], f32)
            nc.vector.tensor_mul(y1, y0, t2)
            t3 = cst.tile([8, 4], f32)
            nc.vector.tensor_mul(t3, y1, y1)
            nc.vector.tensor_mul(t3, t3, ve)
            nc.vector.tensor_scalar(
                out=t3, in0=t3, scalar1=-0.5, scalar2=1.5, op0=Alu.mult, op1=Alu.add
            )
            nc.vector.tensor_mul(musig[:, 1, :], y1, t3)

            # broadcast (mu, rstd) to channels; fold gn affine; xn bf16
            xn_sb = cst.tile([128, 4, T], bf16)
            af = cst.tile([128, 4, 2], f32)
            for i in range(4):
                musig_ps = ps.tile([128, 2], f32, tag="qkv", name=f"musig_ps{i}")
                nc.tensor.matmul(
                    out=musig_ps, lhsT=indT, rhs=musig[:, :, i], start=True, stop=True
                )
                nc.vector.tensor_mul(af[:, i, 0:1], gnw_sb[:, i : i + 1], musig_ps[:, 1:2])
                tmp = work.tile([128, 1], f32, tag="tmp1")
                nc.vector.tensor_mul(tmp, musig_ps[:, 0:1], af[:, i, 0:1])
                nc.vector.tensor_sub(af[:, i, 1:2], gnb_sb[:, i : i + 1], tmp)
                nc.vector.tensor_scalar(
                    out=xn_sb[:, i, :],
                    in0=x_sb[:, i, :],
                    scalar1=af[:, i, 0:1],
                    scalar2=af[:, i, 1:2],
                    op0=Alu.mult,
                    op1=Alu.add,
                )

            # ---------------- QKV machinery ----------------
            q_sb = cst.tile([128, 4, T], bf16)
            k_sb = cst.tile([128, 4, T], bf16)
            vt_sb = cst.tile([128, 8, 8, 65], bf16)
            nc.vector.memset(vt_sb[:, :, :, 64:65], 1.0)

            def emit_qk_boot(j):
                # prologue q_j/k_j through the sc slots, full width
                for oc, dst in ((j, q_sb), (4 + j, k_sb)):
                    qp = ps.tile([128, T], f32, tag="sc", bufs=2, name=f"boot_{oc}")
                    for n in range(2):
                        for i in range(4):
                            nc.tensor.matmul(
                                out=qp[:, 512 * n : 512 * n + 512],
                                lhsT=w1t_sb[:, i, 128 * oc : 128 * oc + 128],
                                rhs=xn_sb[:, i, 512 * n : 512 * n + 512],
                                start=(i == 0),
                                stop=(i == 3),
                            )
                    nc.vector.tensor_scalar(
                        out=dst[:, j, :],
                        in0=qp,
                        scalar1=b1r_sb[:, oc : oc + 1],
                        scalar2=SCALE,
                        op0=Alu.add,
                        op1=Alu.mult,
                    )

            def emit_qk(j):
                # steady-state q_j/k_j in [128,512] halves through the staging slot
                for oc, dst in ((j, q_sb), (4 + j, k_sb)):
                    for n in range(2):
                        qp = ps.tile([128, 512], f32, tag="qkv", name=f"qk_{oc}_{n}")
                        for i in range(4):
                            nc.tensor.matmul(
                                out=qp,
                                lhsT=w1t_sb[:, i, 128 * oc : 128 * oc + 128],
                                rhs=xn_sb[:, i, 512 * n : 512 * n + 512],
                                start=(i == 0),
                                stop=(i == 3),
                            )
                        nc.vector.tensor_scalar(
                            out=dst[:, j, 512 * n : 512 * n + 512],
                            in0=qp,
                            scalar1=b1r_sb[:, oc : oc + 1],
                            scalar2=SCALE,
                            op0=Alu.add,
                            op1=Alu.mult,
                        )

            def emit_v(st):
                vp = ps.tile([128, 512], f32, tag="qkv", name=f"v_{st}")
                for i in range(4):
                    nc.tensor.matmul(
                        out=vp,
                        lhsT=xn_sb[:, i, 128 * st : 128 * st + 128],
                        rhs=w1t_sb[:, i, 2 * C : 3 * C],
                        start=(i == 0),
                        stop=(i == 3),
                    )
                nc.vector.tensor_copy(
                    out=vt_sb[:, st, :, 0:64],
                    in_=vp.rearrange("p (h c) -> p h c", c=64),
                )

            def epilogue(j, h, n, avt):
                rbc = wtp.tile([64, 512], f32, tag="rbc", bufs=4, name=f"rbc_{h}_{n}")
                if EPI_V2:
                    rrow = wtp.tile([1, 512], f32, tag="rrow", bufs=4, name=f"rw_{h}_{n}")
                    nc.vector.tensor_copy(out=rrow, in_=avt[64:65, :])
                    rsp = wtp.tile([64, 8], f32, tag="rsp", bufs=4, name=f"rp_{h}_{n}")
                    nc.sync.dma_start(out=rsp, in_=rrow)
                    rsp2 = wtp.tile([64, 8], f32, tag="rsp2", bufs=4, name=f"rq_{h}_{n}")
                    nc.vector.reciprocal(out=rsp2, in_=rsp)
                    rrow2 = wtp.tile([1, 512], f32, tag="rrow2", bufs=4, name=f"r2_{h}_{n}")
                    nc.sync.dma_start(out=rrow2, in_=rsp2)
                    srcap = rrow2[0:1, :]
                    nc.gpsimd.dma_start(
                        out=rbc,
                        in_=bass.AP(
                            tensor=srcap.tensor,
                            offset=srcap.offset,
                            ap=[srcap.ap[0], [0, 64], srcap.ap[1]],
                        ),
                    )
                else:
                    # 1/r straight off psum row 64; gpsimd broadcasts it
                    rrowi = wtp.tile([1, 512], f32, tag="rrowi", bufs=4, name=f"ri_{h}_{n}")
                    nc.vector.reciprocal_approx_fast(out=rrowi, in_=avt[64:65, :])
                    nc.gpsimd.partition_broadcast(rbc, rrowi)
                o_bf = outp.tile([64, 512], bf16, tag="obf", name=f"ob_{h}_{n}")
                nc.vector.tensor_mul(o_bf, avt[0:64, :], rbc)
                out_f = outp.tile([64, 512], f32, tag="of", name=f"of_{h}_{n}")
                if n == 0 and not EPI_V2:
                    ob2 = outp.tile([64, 512], bf16, tag="ob2", name=f"o2_{h}_{n}")
                    nc.gpsimd.tensor_scalar(
                        out=ob2, in0=o_bf, scalar1=b1vh_sb[:, h : h + 1],
                        scalar2=None, op0=Alu.add,
                    )
                    nc.gpsimd.tensor_add(
                        out_f, ob2, x_hd[:, h, 512 * n : 512 * n + 512]
                    )
                else:
                    nc.vector.scalar_tensor_tensor(
                        out=out_f,
                        in0=o_bf,
                        scalar=b1vh_sb[:, h : h + 1],
                        in1=x_hd[:, h, 512 * n : 512 * n + 512],
                        op0=Alu.add,
                        op1=Alu.add,
                    )
                nc.gpsimd.dma_start(
                    out=out_d[64 * h : 64 * h + 64, 512 * n : 512 * n + 512], in_=out_f
                )

            # ---------------- pipeline ----------------
            emit_qk_boot(0)

            for j in range(HEADS // 2):
                hA, hB = 2 * j, 2 * j + 1
                last = (j == HEADS // 2 - 1) and LAST_INLINE
                av = {
                    (hA, 0): ps.tile([65, 512], f32, tag="av", bufs=3, name=f"av_{hA}_0"),
                    (hB, 0): ps.tile([65, 512], f32, tag="av", bufs=3, name=f"av_{hB}_0"),
                }
                if last:
                    # borrow the idle staging slot for a 4th inline accumulator
                    av[(hA, 1)] = ps.tile([128, 512], f32, tag="qkv", name=f"av_{hA}_1")
                    av[(hB, 1)] = ps.tile([65, 512], f32, tag="av", bufs=3, name=f"av_{hB}_1")
                wts = []
                for st in range(8):
                    if j == 0:
                        emit_v(st)
                    wt_pair = []
                    for n in range(2):
                        scn = ps.tile([128, T], f32, tag="sc", bufs=2, name=f"sc_{j}_{st}_{n}")
                        for hi, h in enumerate((hA, hB)):
                            hp = 64 * hi
                            nc.tensor.matmul(
                                out=scn[:, 512 * hi : 512 * hi + 512],
                                lhsT=k_sb[hp : hp + 64, j, 128 * st : 128 * st + 128],
                                rhs=q_sb[hp : hp + 64, j, 512 * n : 512 * n + 512],
                                start=True,
                                stop=True,
                                tile_position=(hp, 0),
                            )
                        wtn = wtp.tile(
                            [128, T], bf16, tag="wt", bufs=20, name=f"wt_{j}_{st}_{n}"
                        )
                        if (st, n) in DVE_EXP_UNITS:
                            nc.vector.tensor_scalar(
                                out=wtn.bitcast(i16),
                                in0=scn,
                                scalar1=LOG2E_128,
                                scalar2=SCHRAUD_B,
                                op0=Alu.mult,
                                op1=Alu.add,
                            )
                        else:
                            nc.scalar.activation(
                                out=wtn, in_=scn, func=Act.Exp, bias=0.0, scale=1.0
                            )
                        wt_pair.append(wtn)
                    wts.append(wt_pair)

                    # AV (n=0 inline; n=1 too on the last pair)
                    n_range = (0, 1) if last else (0,)
                    for n in n_range:
                        for hi, h in enumerate((hA, hB)):
                            nc.tensor.matmul(
                                out=av[(h, n)][0:65, :],
                                lhsT=vt_sb[:, st, h, 0:65],
                                rhs=wt_pair[n][:, 512 * hi : 512 * hi + 512],
                                start=(st == 0),
                                stop=(st == 7),
                            )

                if not last:
                    if j < HEADS // 2 - 1:
                        emit_qk(j + 1)
                    epilogue(j, hA, 0, av[(hA, 0)])
                    av[(hA, 1)] = ps.tile([65, 512], f32, tag="av", bufs=3, name=f"av_{hA}_1")
                    epilogue(j, hB, 0, av[(hB, 0)])
                    av[(hB, 1)] = ps.tile([65, 512], f32, tag="av", bufs=3, name=f"av_{hB}_1")
                    for st in range(8):
                        for hi, h in enumerate((hA, hB)):
                            nc.tensor.matmul(
                                out=av[(h, 1)][0:65, :],
                                lhsT=vt_sb[:, st, h, 0:65],
                                rhs=wts[st][1][:, 512 * hi : 512 * hi + 512],
                                start=(st == 0),
                                stop=(st == 7),
                            )
                    epilogue(j, hA, 1, av[(hA, 1)])
                    epilogue(j, hB, 1, av[(hB, 1)])
                else:
                    for h in (hA, hB):
                        for n in (0, 1):
                            epilogue(j, h, n, av[(h, n)])

    nc.finalize()
    return nc


def _make_in_maps(inputs):
    x = np.ascontiguousarray(np.asarray(inputs["x"], dtype=np.float32))
    gnw = np.asarray(inputs["gn_weight"], dtype=np.float32)
    gnb = np.asarray(inputs["gn_bias"], dtype=np.float32)
    w1 = np.asarray(inputs["w1"], dtype=np.float32)
    b1 = np.asarray(inputs["b1"], dtype=np.float32)

    import ml_dtypes

    B = x.shape[0]
    w1t = np.ascontiguousarray(w1[:, :, 0].T).astype(ml_dtypes.bfloat16)  # [C, 3C]
    b1r = np.ascontiguousarray(b1[: 2 * C].reshape(8, 128).T)       # [128, 8]
    b1vh = np.ascontiguousarray(b1[2 * C :].reshape(8, 64).T)       # [64, 8]
    gnw_r = np.ascontiguousarray(gnw.reshape(4, 128).T)             # [128, 4]
    gnb_r = np.ascontiguousarray(gnb.reshape(4, 128).T)             # [128, 4]

    ind16 = np.zeros((128, 8), np.float32)
    indT = np.zeros((8, 128), np.float32)
    for g in range(8):
        ind16[16 * g : 16 * g + 16, g] = 1.0 / 16.0
        indT[g, 16 * g : 16 * g + 16] = 1.0

    in_maps = []
    for b in range(B):
        in_maps.append(
            {
                "x": np.ascontiguousarray(x[b].reshape(C, T)),
                "w1t": w1t,
                "b1r": b1r,
                "b1vh": b1vh,
                "gnw": gnw_r,
                "gnb": gnb_r,
                "ind16": ind16,
                "indT": indT,
            }
        )
    return in_maps


def _gather(results, x_shape):
    B, Cc, H, W = x_shape
    out = np.empty((B, Cc, H, W), dtype=np.float32)
    for b in range(B):
        out[b] = results[b]["out"].reshape(Cc, H, W)
    return out


def kernel(**inputs):
    from concourse.bass_utils import run_bass_kernel_spmd

    nc = _build_nc()
    in_maps = _make_in_maps(inputs)
    res = run_bass_kernel_spmd(nc, in_maps, core_ids=list(range(N_CORES)))
    return _gather(res.results, np.asarray(inputs["x"]).shape)
